# revision 2
# baseline (speedup 1.0000x reference)
"""Trainium2 Bass kernel for nn_Colorizer (retrieval_knn).

Pipeline (per sample, data-parallel over N=8 samples -> 8 cores):
  1. Patch-embed conv as matmul: featsT[c, p] = W[k, c]^T @ patchesT[k, p]
     (k = 8*8*3 = 192 patch pixels padded to 256, p = 4096 patches)
  2. Similarity S[r, t] = refT[c, r]^T @ tgtT[c, t]   (r = 3072, t = 1024)
  3. E = exp(S - 50)  (softmax over r is shift-invariant; max|S| ~= 87 so
     the constant shift prevents fp32 exp overflow; underflow to 0 is safe)
  4. predT_unnorm = labels_aug^T @ E with labels_aug = [ones(16),
     zeros(16), labels(16)]: rows 0..15 = replicated softmax
     denominator, rows 32..47 = unnormalized predictions
  5. Normalize: out = pred_rows * reciprocal(denom_rows), DMA out as
     [16, 1024]; host transposes to [1024, 16].

All tensors flow in bf16 (empirically 5.5e-3 rel err vs the 2e-2 gate;
matmul accumulation stays fp32 in PSUM). Host side only reshapes /
transposes / casts dtypes; all FLOPs run on device.

Perf notes (v2, trace-driven):
  - inputs split across THREE DMA rings (sync + scalar HWDGE, gpsimd
    SWDGE); labels go dense (768B packets) + on-chip scatter instead of
    a strided 64B-packet DMA that clogged the ring.
  - warm-up matmuls (HAM clock ramp 1.2 -> 2.4 GHz) depend only on one
    tiny DVE memset so they start right after the NEFF preamble.
  - conv blocks 1/2 are interleaved into the similarity stream so the
    PSUM->SBUF casts (all on DVE) never gate the PE.
  - last exp is split in halves so the final pred matmuls overlap it.
"""

import numpy as np
import ml_dtypes

import concourse.mybir as mybir
from concourse import bacc
from concourse.bass_utils import run_bass_kernel_spmd
from concourse.tile import TileContext

F32 = mybir.dt.float32
BF16 = mybir.dt.bfloat16
BF16_NP = ml_dtypes.bfloat16

N = 8            # samples == cores
R_T, T_T = 3, 1  # ref / target frames
H = W_IMG = 256
C = 3
PATCH = 8
FEAT = 256
K_LAB = 16
HP = H // PATCH          # 32
PPI = HP * HP            # 1024 patches per image
NIMG = R_T + T_T         # 4
NPAT = NIMG * PPI        # 4096
KPIX = PATCH * PATCH * C  # 192
KPAD = 256               # K padded to 2x128
R = R_T * PPI            # 3072
T = T_T * PPI            # 1024
RC = R // 128            # 24 r-chunks
LABC = 48                # 16 ones cols, 16 zero cols, 16 label cols
EXP_SHIFT = -50.0
N_WARMUP = 8
NB = 4                   # conv column blocks (one per image)
BW = NPAT // NB          # 1024


def _build_nc():
    nc = bacc.Bacc(trn_type="TRN2", target_bir_lowering=False)

    pt_d = nc.declare_dram_parameter("pt", [KPIX, NPAT], BF16, isOutput=False)
    w_d = nc.declare_dram_parameter("w", [KPAD, FEAT], BF16, isOutput=False)
    lab_d = nc.declare_dram_parameter("lab", [128, RC * K_LAB], BF16, isOutput=False)
    out_d = nc.declare_dram_parameter("out", [K_LAB, T], F32, isOutput=True)

    with TileContext(nc) as tc:
        with (
            tc.tile_pool(name="const", bufs=1) as const,
            tc.tile_pool(name="feats", bufs=1) as feats,
            tc.tile_pool(name="cps", bufs=2, space="PSUM") as cps,
            tc.tile_pool(name="sps", bufs=2, space="PSUM") as sps,
            tc.tile_pool(name="predps", bufs=1, space="PSUM") as predps,
            tc.tile_pool(name="epool", bufs=4) as epool,
            tc.tile_pool(name="opool", bufs=2) as opool,
        ):
            # PE warm-up source: one tiny DVE memset so matmuls start early
            wu_sb = const.tile([128, 512], BF16, tag="wu")
            nc.vector.memset(wu_sb, 0.0)

            # ---- input loads: 3 DMA rings, tgt image (block 3) first ----
            w_sb0 = const.tile([128, FEAT], BF16, tag="w0")
            w_sb1 = const.tile([128, FEAT], BF16, tag="w1")
            pt_sb0 = const.tile([128, NPAT], BF16, tag="pt0")
            pt_sb1 = const.tile([128, NPAT], BF16, tag="pt1")
            lab_stage = const.tile([128, RC, K_LAB], BF16, tag="lst")

            shift_sb = const.tile([128, 1], F32, tag="shift")
            # zero-pad rows 64:128 of the K=128..255 rhs tile on-chip
            # (lhsT rows there are zero, but 0*garbage-NaN would poison PSUM)
            nc.vector.memset(pt_sb1[64:128, :], 0.0)
            nc.vector.memset(shift_sb, EXP_SHIFT)

            NB_ORDER = (3, 0, 1, 2)  # tgt image block first
            # ring A (sync HWDGE): w0 + left 512-col half of each pt0 block
            nc.sync.dma_start(out=w_sb0, in_=w_d.ap()[0:128, :])
            for nb in NB_ORDER:
                sl = slice(nb * BW, nb * BW + 512)
                nc.sync.dma_start(out=pt_sb0[:, sl], in_=pt_d.ap()[0:128, sl])
            # ring B (scalar HWDGE): w1 + right halves of pt0 blocks
            nc.scalar.dma_start(out=w_sb1[0:KPIX - 128, :],
                                in_=w_d.ap()[128:KPIX, :])
            nc.scalar.dma_start(out=w_sb1[KPIX - 128:128, :],
                                in_=w_d.ap()[KPIX:KPAD, :])
            for nb in NB_ORDER:
                sl = slice(nb * BW + 512, (nb + 1) * BW)
                nc.scalar.dma_start(out=pt_sb0[:, sl], in_=pt_d.ap()[0:128, sl])
            # ring C (gpsimd SWDGE): pt1 blocks + labels (dense)
            for nb in NB_ORDER:
                sl = slice(nb * BW, (nb + 1) * BW)
                nc.gpsimd.dma_start(
                    out=pt_sb1[0:KPIX - 128, sl], in_=pt_d.ap()[128:KPIX, sl]
                )
            nc.gpsimd.dma_start(
                out=lab_stage,
                in_=lab_d.ap().rearrange("p (rc k) -> p rc k", k=K_LAB),
            )

            # labels_aug: [ones | zeros | labels] built on-chip
            lab_sb = const.tile([128, RC, LABC], BF16, tag="lab")
            nc.gpsimd.memset(lab_sb[:, :, 0:16], 1.0)
            nc.gpsimd.memset(lab_sb[:, :, 16:32], 0.0)
            nc.vector.tensor_copy(lab_sb[:, :, 32:48], lab_stage)

            # ---- PE clock warm-up during the DMA prologue (HAM) ----
            for _ in range(N_WARMUP):
                wps = cps.tile([128, 512], F32, tag="cp", name="wps")
                nc.tensor.matmul(wps, wu_sb[:, 0:128], wu_sb, start=True, stop=True)

            # ---- 1. conv: featsT[c, p] (c split in two 128-row tiles) ----
            f_sb = [
                feats.tile([128, NPAT], BF16, tag="f0", name="f_sb0"),
                feats.tile([128, NPAT], BF16, tag="f1", name="f_sb1"),
            ]
            pred_ps = predps.tile([LABC, T], F32, tag="pred")

            def conv_unit(nb, h, cc):
                ps = cps.tile([128, 512], F32, tag="cp", name="ps")
                csl = slice(cc * 128, (cc + 1) * 128)
                hsl = slice(nb * BW + h * 512, nb * BW + (h + 1) * 512)
                nc.tensor.matmul(ps, w_sb0[:, csl], pt_sb0[:, hsl],
                                 start=True, stop=False)
                nc.tensor.matmul(ps, w_sb1[:, csl], pt_sb1[:, hsl],
                                 start=False, stop=True)
                nc.vector.tensor_copy(f_sb[cc][:, hsl], ps)

            def conv_block(nb):
                for h in range(2):
                    for cc in range(2):
                        conv_unit(nb, h, cc)

            e_tiles = {}

            def s_part(rc, split_last=False):
                rsl = slice(rc * 128, (rc + 1) * 128)
                s_ps = sps.tile([128, T], F32, tag="s", name="s_ps")
                for cc in range(2):
                    for th in range(2):
                        psl = slice(th * 512, (th + 1) * 512)
                        tsl = slice(R + th * 512, R + (th + 1) * 512)
                        nc.tensor.matmul(
                            s_ps[:, psl], f_sb[cc][:, rsl], f_sb[cc][:, tsl],
                            start=(cc == 0), stop=(cc == 1),
                        )
                e_sb = epool.tile([128, T], BF16, tag="e", name="e_sb")
                if split_last:
                    for th in range(2):
                        psl = slice(th * 512, (th + 1) * 512)
                        nc.scalar.activation(
                            e_sb[:, psl], s_ps[:, psl],
                            mybir.ActivationFunctionType.Exp,
                            bias=shift_sb, scale=1.0,
                        )
                else:
                    nc.scalar.activation(
                        e_sb, s_ps, mybir.ActivationFunctionType.Exp,
                        bias=shift_sb, scale=1.0,
                    )
                e_tiles[rc] = e_sb

            def pred_part(rc):
                e_sb = e_tiles.pop(rc)
                for th in range(2):
                    psl = slice(th * 512, (th + 1) * 512)
                    nc.tensor.matmul(
                        pred_ps[:, psl],
                        lab_sb[:, rc, :],
                        e_sb[:, psl],
                        start=(rc == 0), stop=(rc == RC - 1),
                    )

            # conv(3) + conv(0) up front; conv(1)/conv(2) units are
            # interleaved into the similarity stream; pred lags 2 chunks
            # behind its exp so the PE never waits on ACT.
            conv_block(3)
            conv_block(0)
            CU1 = {1: (1, 0, 0), 2: (1, 0, 1), 3: (1, 1, 0), 4: (1, 1, 1)}
            CU2 = {9: (2, 0, 0), 10: (2, 0, 1), 11: (2, 1, 0), 12: (2, 1, 1)}
            for rc in range(RC):
                s_part(rc, split_last=(rc == RC - 1))
                if rc >= 2:
                    pred_part(rc - 2)
                cu = CU1.get(rc) or CU2.get(rc)
                if cu:
                    conv_unit(*cu)
            pred_part(RC - 2)
            pred_part(RC - 1)

            # ---- 5. normalize label rows by replicated denom rows ----
            rec = opool.tile([K_LAB, T], F32, tag="rec")
            nc.vector.reciprocal_approx_fast(rec, pred_ps[0:K_LAB, :])
            o_sb = opool.tile([K_LAB, T], F32, tag="o")
            nc.vector.tensor_mul(o_sb, pred_ps[32:32 + K_LAB, :], rec)
            nc.sync.dma_start(out=out_d.ap(), in_=o_sb)

    nc.compile()
    return nc


_NC_CACHE = None


def _get_nc():
    global _NC_CACHE
    if _NC_CACHE is None:
        _NC_CACHE = _build_nc()
    return _NC_CACHE


def prep_in_maps(reference_images, target_images, reference_labels, w_feat):
    """Host-side sharding + layout prep (reshape/transpose/dtype only)."""
    ri = np.ascontiguousarray(reference_images, dtype=np.float32)
    ti = np.ascontiguousarray(target_images, dtype=np.float32)
    lab = np.ascontiguousarray(reference_labels, dtype=np.float32)
    wf = np.ascontiguousarray(w_feat, dtype=np.float32)

    w2 = np.zeros((KPAD, FEAT), np.float32)
    w2[:KPIX] = wf.reshape(KPIX, FEAT)
    w2 = w2.astype(BF16_NP)
    imgs = np.concatenate([ri, ti], axis=1)  # [N, 4, H, W, C]
    # patchesT[n] : [(dy dx ch), (img py px)]
    ptT = np.ascontiguousarray(
        imgs.reshape(N, NIMG, HP, PATCH, HP, PATCH, C)
        .transpose(0, 3, 5, 6, 1, 2, 4)
        .reshape(N, KPIX, NPAT)
        .astype(BF16_NP)
    )
    lab_sw = np.ascontiguousarray(
        lab.reshape(N, RC, 128, K_LAB).transpose(0, 2, 1, 3)
        .reshape(N, 128, RC * K_LAB)
        .astype(BF16_NP)
    )
    return [
        {"pt": ptT[n], "w": w2, "lab": lab_sw[n]} for n in range(N)
    ]


def run(in_maps, **kwargs):
    nc = _get_nc()
    return run_bass_kernel_spmd(nc, in_maps, list(range(N)), **kwargs)


def kernel(reference_images, target_images, reference_labels, w_feat):
    in_maps = prep_in_maps(
        reference_images, target_images, reference_labels, w_feat
    )
    res = run(in_maps)
    # device emits [16, T]; transpose to [T, 16] here (pure layout)
    out = np.stack(
        [np.ascontiguousarray(res.results[n]["out"].T) for n in range(N)]
    )
    return out.reshape(N, T_T, HP, HP, K_LAB)
